# revision 2
# baseline (speedup 1.0000x reference)
"""Trainium2 Bass kernel v4: decoder layer, ROW-parallel SPMD over 8 cores.

Core c = (batch b = c//2, row-set c%2). Row-sets are causal-balanced
interleaved 128-row slices: set0 = {0,3,4,7,8,11,12,15}, set1 = the rest.
Each core computes ALL 8 heads (averaged) for its own 1024 rows -> no
collectives. Host precomputes LN1 and folds Wq@Wk^T per head (E==D) so K
projections vanish; big matmuls run fp8 e4m3 with DoubleRow (contract 256).

One SPMD program for all cores: causal loop bounds are unified by padding
slice i to 2*(i+1) key tiles; the (full / diagonal / invalid) structure of
the boundary tile kt is applied as a data-driven mask (ones / tri / zero)
uploaded per core, one mask multiply per key tile.

Scales (powers of 2, folded into activation scale/bias):
  xn1/k/v/z fp8 = 16*x ; M~ = 8192*M ; q_t = 1024*(z@M) (evac 1/128)
  scores psum = 16384*s ; exp bias += ln(8) -> p8 = 8*exp(s) fp8
  Wv~ = 1024*Wv ; v_t = 16*V (evac 1/1024) ; rowsum ones = 128
  po/pr = sum(p V)/(8*rowsum) = the per-head mean term, exactly.
  W1u = 1024*(g3 W1) ; h1 = 16*relu(.) (evac 1/1024, bias 16*c1)
  W2u = 1024*W2 ; h2 = psum/16384.
"""
import sys
import types

sys.path.insert(0, "/opt/trn_rl_repo")

import numpy as np
import ml_dtypes

import concourse.bacc as bacc
import concourse.tile as tile
from concourse import mybir
from concourse.bass_utils import run_bass_kernel_spmd

FP8 = ml_dtypes.float8_e4m3
BF16 = ml_dtypes.bfloat16
B, L, E, H, D, FW = 4, 2048, 512, 8, 512, 4
HID = FW * E
P = 128
EC = E // P          # 4
KT = L // P          # 16
HC = HID // P        # 16
OWN = 8              # own row slices per core
OWNL = OWN * P       # 1024
SQ = float(D) ** -0.5
EPS = 1e-5
LN8 = float(np.log(8.0))
ROWSETS = [[0, 3, 4, 7, 8, 11, 12, 15], [1, 2, 5, 6, 9, 10, 13, 14]]

TRACE = False
_CACHE = {}
DR = mybir.MatmulPerfMode.DoubleRow


def _build():
    f32, bf16, fp8 = mybir.dt.float32, mybir.dt.bfloat16, mybir.dt.float8e4
    nc = bacc.Bacc(None, target_bir_lowering=False, debug=False)

    xf_in = nc.dram_tensor("xn1t_full", [E, L], fp8, kind="ExternalInput")
    xo_in = nc.dram_tensor("xn1t_own", [E, OWNL], fp8, kind="ExternalInput")
    kt_in = nc.dram_tensor("kT", [E, L], fp8, kind="ExternalInput")
    vt_in = nc.dram_tensor("vT", [E, L], fp8, kind="ExternalInput")
    qr_in = nc.dram_tensor("qres", [OWNL, E], f32, kind="ExternalInput")
    ms_in = nc.dram_tensor("m_s", [H, E, E], fp8, kind="ExternalInput")
    mc_in = nc.dram_tensor("m_c", [H, E, E], fp8, kind="ExternalInput")
    wvs_in = nc.dram_tensor("wv_s", [H, E, D], fp8, kind="ExternalInput")
    wvc_in = nc.dram_tensor("wv_c", [H, E, D], fp8, kind="ExternalInput")
    w1_in = nc.dram_tensor("w1u", [E, HID], bf16, kind="ExternalInput")
    w2_in = nc.dram_tensor("w2u", [HID, E], bf16, kind="ExternalInput")
    beta_in = nc.dram_tensor("beta", [P, H * KT], f32, kind="ExternalInput")
    c1_in = nc.dram_tensor("c1s", [P, HC], f32, kind="ExternalInput")
    b2_in = nc.dram_tensor("b2rep", [P, E], f32, kind="ExternalInput")
    id_in = nc.dram_tensor("identity", [P, P], bf16, kind="ExternalInput")
    mk_in = nc.dram_tensor("masks", [P, KT, P], fp8, kind="ExternalInput")
    out_d = nc.dram_tensor("out", [OWNL, E], f32, kind="ExternalOutput")

    with tile.TileContext(nc) as tc:
        with (
            tc.tile_pool(name="consts", bufs=1) as consts,
            tc.tile_pool(name="xin", bufs=1) as xin,
            tc.tile_pool(name="accp", bufs=1) as accp,
            tc.tile_pool(name="wpool", bufs=2) as wpool,
            tc.tile_pool(name="ffw", bufs=1) as ffw,
            tc.tile_pool(name="qtp", bufs=2) as qtp,
            tc.tile_pool(name="vtp", bufs=3) as vtp,
            tc.tile_pool(name="ptp", bufs=2) as ptp,
            tc.tile_pool(name="ztp", bufs=1) as ztp,
            tc.tile_pool(name="h1p", bufs=1) as h1p,
            tc.tile_pool(name="stats", bufs=6) as statp,
            tc.tile_pool(name="tmps", bufs=3) as tmpp,
            tc.tile_pool(name="ps_big", bufs=4, space="PSUM") as psb,
            tc.tile_pool(name="ps_sum", bufs=2, space="PSUM") as pss,
            tc.tile_pool(name="ps_tr", bufs=2, space="PSUM") as pstr,
        ):
            # inputs needed first (order sets DMA priority)
            xft = xin.tile([P, EC, L], fp8)
            nc.sync.dma_start(
                xft[:, :, 0:1024],
                xf_in.rearrange("(c p) t -> p c t", p=P)[:, :, 0:1024])
            xot = xin.tile([P, EC, OWNL], fp8)
            nc.sync.dma_start(xot, xo_in.rearrange("(c p) t -> p c t", p=P))
            ones2 = consts.tile([P, 2, 1], fp8)
            nc.vector.memset(ones2, 128.0)
            ln8t = consts.tile([P, 1], f32)
            nc.vector.memset(ln8t, LN8)
            epsA = consts.tile([P, 1], f32)
            nc.vector.memset(epsA, EPS)
            epsB = consts.tile([P, 1], f32)
            nc.vector.memset(epsB, EPS / 256.0)
            nc.sync.dma_start(
                xft[:, :, 1024:2048],
                xf_in.rearrange("(c p) t -> p c t", p=P)[:, :, 1024:2048])

            acc = accp.tile([P, OWN, E], f32)
            z2T = ztp.tile([P, EC, OWNL], fp8, tag="z2")
            z3T = ztp.tile([P, EC, OWNL], bf16, tag="z3")

            def ln_to_zT(src_ap, zT, col, s16=True):
                st = statp.tile([P, 6], f32, tag="bnst")
                nc.vector.bn_stats(st, src_ap)
                mv = statp.tile([P, 2], f32, tag="bnmv")
                nc.vector.bn_aggr(mv, st)
                sd = statp.tile([P, 1], f32, tag="bnsd")
                nc.scalar.activation(
                    sd, mv[:, 1:2], mybir.ActivationFunctionType.Sqrt,
                    bias=epsB if s16 else epsA,
                    scale=1.0 / 256.0 if s16 else 1.0,
                )
                r16 = statp.tile([P, 1], f32, tag="bnrs")
                nc.vector.reciprocal(r16, sd)
                xn = tmpp.tile([P, E], bf16, tag="bfs")
                nc.vector.tensor_scalar(
                    out=xn, in0=src_ap, scalar1=mv[:, 0:1], scalar2=r16,
                    op0=mybir.AluOpType.subtract, op1=mybir.AluOpType.mult,
                )
                for dt in range(EC):
                    pt = pstr.tile([P, P], bf16, tag="ptr")
                    nc.tensor.transpose(pt, xn[:, dt * P:(dt + 1) * P], idt)
                    nc.scalar.activation(
                        zT[:, dt, col:col + P], pt,
                        mybir.ActivationFunctionType.Copy,
                    )

            def vproj_kt(kt, vkeyT, wv, v_t):
                ps = psb.tile([P, 512], f32, tag="psb")
                for c2 in range(EC // 2):
                    nc.tensor.matmul(
                        ps,
                        vkeyT[:, 2 * c2:2 * c2 + 2, kt * P:(kt + 1) * P],
                        wv[:, 2 * c2:2 * c2 + 2, :],
                        start=(c2 == 0), stop=(c2 == EC // 2 - 1),
                        perf_mode=DR,
                    )
                nc.vector.tensor_scalar_mul(v_t[:, kt, :], ps, 1.0 / 1024.0)

            def vproj_weights(h, wv_d):
                wv = wpool.tile([P, EC, D], fp8, tag="wv")
                nc.sync.dma_start(wv, wv_d[h].rearrange("(c p) d -> p c d", p=P))
                return wv

            def projq_units(h, xqT, m_d):
                """Thunk list: 8 psum groups of the Q~ projection + DVE evacs."""
                wm = wpool.tile([P, EC, E], fp8, tag="wm")
                nc.sync.dma_start(wm, m_d[h].rearrange("(c p) e -> p c e", p=P))
                q_t = qtp.tile([P, EC, OWNL], fp8, tag="qt")

                def unit(rb, e2):
                    def f():
                        ps = psb.tile([P, 512], f32, tag="psb")
                        for c2 in range(EC // 2):
                            nc.tensor.matmul(
                                ps,
                                wm[:, 2 * c2:2 * c2 + 2, e2 * P:(e2 + 1) * P],
                                xqT[:, 2 * c2:2 * c2 + 2,
                                    rb * 512:(rb + 1) * 512],
                                start=(c2 == 0), stop=(c2 == EC // 2 - 1),
                                perf_mode=DR,
                            )
                        nc.vector.tensor_scalar_mul(
                            q_t[:, e2, rb * 512:(rb + 1) * 512], ps, 1.0 / 128.0)
                    return f
                return q_t, [unit(rb, e2) for rb in range(2) for e2 in range(EC)]

            def vproj_units(h, vkeyT, wv_d):
                wv = vproj_weights(h, wv_d)
                v_t = vtp.tile([P, KT, D], fp8, tag="vt")
                return v_t, [
                    (lambda kt: lambda: vproj_kt(kt, vkeyT, wv, v_t))(kt)
                    for kt in range(KT)
                ]

            def sc_units(h, q_t, keyT, causal, bcol):
                p_t = ptp.tile([P, KT, OWNL], fp8, tag="pt")

                def unit(kt, cs, ce, mask):
                    def f():
                        ps = psb.tile([P, 512], f32, tag="psb")
                        for c2 in range(EC // 2):
                            nc.tensor.matmul(
                                ps[:, 0:ce - cs],
                                keyT[:, 2 * c2:2 * c2 + 2, kt * P:(kt + 1) * P],
                                q_t[:, 2 * c2:2 * c2 + 2, cs:ce],
                                start=(c2 == 0), stop=(c2 == EC // 2 - 1),
                                perf_mode=DR,
                            )
                        bias = (
                            betat[:, bcol + kt:bcol + kt + 1]
                            if bcol is not None else ln8t
                        )
                        nc.scalar.activation(
                            p_t[:, kt, cs:ce], ps[:, 0:ce - cs],
                            mybir.ActivationFunctionType.Exp,
                            bias=bias, scale=1.0 / 16384.0,
                        )
                        if mask:
                            i = kt // 2
                            nc.gpsimd.tensor_mul(
                                p_t[:, kt, i * P:(i + 1) * P],
                                p_t[:, kt, i * P:(i + 1) * P],
                                masks[:, kt, :],
                            )
                    return f

                units = []
                for kt in range(KT):
                    c0 = (kt // 2) * P if causal else 0
                    first = True
                    for cs in range(c0, OWNL, 512):
                        ce = min(cs + 512, OWNL)
                        units.append(unit(kt, cs, ce, causal and first))
                        first = False
                return p_t, units

            def emit_interleaved(a, b):
                """Emit thunks from a and b, spreading b evenly among a."""
                if not b:
                    for f in a:
                        f()
                    return
                ratio = max(1, len(a) // len(b))
                bi = 0
                for n, f in enumerate(a):
                    f()
                    if n % ratio == ratio - 1 and bi < len(b):
                        b[bi]()
                        bi += 1
                while bi < len(b):
                    b[bi]()
                    bi += 1

            def av(p_t, v_t, causal, post_slice=None):
                for i in range(OWN):
                    pairs = (i + 1) if causal else KT // 2
                    cs = slice(i * P, (i + 1) * P)
                    pr = pss.tile([P, 1], f32, tag="pssum")
                    for j in range(pairs):
                        nc.tensor.matmul(
                            pr, p_t[:, 2 * j:2 * j + 2, cs], ones2[:, :, :],
                            start=(j == 0), stop=(j == pairs - 1),
                            perf_mode=DR,
                        )
                    r8 = statp.tile([P, 1], f32, tag="r8")
                    nc.vector.reciprocal(r8, pr)
                    po = psb.tile([P, 512], f32, tag="psb")
                    for j in range(pairs):
                        nc.tensor.matmul(
                            po, p_t[:, 2 * j:2 * j + 2, cs],
                            v_t[:, 2 * j:2 * j + 2, :],
                            start=(j == 0), stop=(j == pairs - 1),
                            perf_mode=DR,
                        )
                    ot = tmpp.tile([P, E], f32, tag="f32s")
                    nc.vector.tensor_scalar_mul(ot, po, r8)
                    nc.gpsimd.tensor_add(acc[:, i, :], acc[:, i, :], ot)
                    if post_slice is not None:
                        post_slice(i)

            # ================= self-attention, software-pipelined =================
            v0, vu0 = vproj_units(0, xft, wvs_in)
            q0, qu0 = projq_units(0, xot, ms_in)
            emit_interleaved(vu0, [])
            emit_interleaved(qu0, [])
            # constants not needed in the first ~10us
            idt = consts.tile([P, P], bf16)
            nc.sync.dma_start(idt, id_in[:, :])
            masks = consts.tile([P, KT, P], fp8)
            nc.sync.dma_start(masks, mk_in[:, :, :])
            betat = consts.tile([P, H * KT], f32)
            nc.sync.dma_start(betat, beta_in[:, :])
            c1t = consts.tile([P, HC], f32)
            nc.sync.dma_start(c1t, c1_in[:, :])
            b2t = consts.tile([P, E], f32)
            nc.sync.dma_start(b2t, b2_in[:, :])
            nc.sync.dma_start(acc, qr_in.rearrange("(s p) e -> p s e", p=P))

            v_prev, q_prev = v0, q0
            p_prev = None
            for h in range(H):
                # interleave scores(h) with next head's projections
                pu_next = []
                if h + 1 < H:
                    v_next, vun = vproj_units(h + 1, xft, wvs_in)
                    q_next, qun = projq_units(h + 1, xot, ms_in)
                    pu_next = vun + qun
                else:
                    # last self head: overlap with cross head0 V-projection
                    v_next, vun = vproj_units(0, vtt, wvc_in)
                    q_next = None
                    pu_next = vun
                p_h, scu = sc_units(h, q_prev, xft, True, None)
                emit_interleaved(scu, pu_next)
                if p_prev is not None:
                    av(p_prev, vv_prev, True)
                p_prev, vv_prev = p_h, v_prev
                v_prev, q_prev = v_next, q_next
                if h == 1:
                    ktt = xin.tile([P, EC, L], fp8)
                    nc.sync.dma_start(
                        ktt, kt_in.rearrange("(c p) t -> p c t", p=P))
                    vtt = xin.tile([P, EC, L], fp8)
                    nc.sync.dma_start(
                        vtt, vt_in.rearrange("(c p) t -> p c t", p=P))
                elif h == 2:
                    w1t = ffw.tile([P, EC, HID], bf16)
                    nc.sync.dma_start(
                        w1t, w1_in.rearrange("(c p) d -> p c d", p=P))
                    w2t = ffw.tile([P, HC, E], bf16)
                    nc.sync.dma_start(
                        w2t, w2_in.rearrange("(c p) d -> p c d", p=P))
            av(p_prev, vv_prev, True)   # self head 7

            # ====== LN2 -> z2T, interleaved with cross head1 V-proj ======
            vc1, vuc1 = vproj_units(1, vtt, wvc_in)
            for i in range(OWN):
                ln_to_zT(acc[:, i, :], z2T, i * P)
                for j in range(2 * i, 2 * i + 2):
                    vuc1[j]()

            # ================= cross-attention, software-pipelined =================
            vs = {0: v_prev, 1: vc1}          # v_prev = cross head0 v_t
            q_prev, _qu = projq_units(0, z2T, mc_in)
            emit_interleaved(_qu, [])
            p_prev = None
            vv_prev = None
            for h in range(H):
                pu_next = []
                if h + 1 < H:
                    if h >= 1 and h + 1 not in vs:
                        vs[h + 1], vun = vproj_units(h + 1, vtt, wvc_in)
                        pu_next += vun
                    q_next, qun = projq_units(h + 1, z2T, mc_in)
                    pu_next += qun
                else:
                    q_next = None
                p_h, scu = sc_units(h, q_prev, ktt, False, h * KT)
                emit_interleaved(scu, pu_next)
                if p_prev is not None:
                    av(p_prev, vv_prev, False)
                p_prev, vv_prev = p_h, vs[h]
                q_prev = q_next

            # last cross AV with LN3 + b2 interleaved per slice
            def post_ln3(i):
                ln_to_zT(acc[:, i, :], z3T, i * P, s16=False)
                nc.vector.tensor_add(acc[:, i, :], acc[:, i, :], b2t)
            av(p_prev, vv_prev, False, post_slice=post_ln3)

            # ================= FFN per row-block =================
            for rb in range(2):
                h1t = h1p.tile([P, HC, 512], bf16, tag="h1")
                for hc in range(HC):
                    ps = psb.tile([P, 512], f32, tag="psb")
                    for c2 in range(EC):
                        nc.tensor.matmul(
                            ps,
                            w1t[:, c2, hc * P:(hc + 1) * P],
                            z3T[:, c2, rb * 512:(rb + 1) * 512],
                            start=(c2 == 0), stop=(c2 == EC - 1),
                        )
                    nc.scalar.activation(
                        h1t[:, hc, :], ps,
                        mybir.ActivationFunctionType.Relu,
                        bias=c1t[:, hc:hc + 1],
                    )
                for i in range(rb * 4, rb * 4 + 4):
                    io = i - rb * 4
                    ps = psb.tile([P, 512], f32, tag="psb")
                    for j in range(HC):
                        nc.tensor.matmul(
                            ps,
                            h1t[:, j, io * P:(io + 1) * P],
                            w2t[:, j, :],
                            start=(j == 0), stop=(j == HC - 1),
                        )
                    t1 = tmpp.tile([P, E], f32, tag="f32s")
                    nc.scalar.activation(
                        t1, ps, mybir.ActivationFunctionType.Copy,
                    )
                    t2 = tmpp.tile([P, E], f32, tag="f32o")
                    nc.vector.tensor_add(t2, t1, acc[:, i, :])
                    nc.sync.dma_start(out_d[i * P:(i + 1) * P, :], t2)

    nc.compile()
    return nc


def _q8(x, scale=1.0):
    return np.ascontiguousarray(
        np.clip(np.asarray(x, np.float32) * scale, -240, 240).astype(FP8)
    )


def _ensure_ntff_hook():
    try:
        from antenv.axon_hooks import get_axon_ntff_profile_hook  # noqa: F401
        return
    except ImportError:
        pass
    import antenv

    mod = types.ModuleType("antenv.axon_hooks")
    _hook = [None]
    mod.set_axon_ntff_profile_hook = lambda h: _hook.__setitem__(0, h)
    mod.get_axon_ntff_profile_hook = lambda: _hook[0]
    sys.modules["antenv.axon_hooks"] = mod
    antenv.axon_hooks = mod
    from trn_agent_boot.trn_boot import _ntff_profile_via_ctypes

    mod.set_axon_ntff_profile_hook(
        _ntff_profile_via_ctypes("/opt/axon/libaxon_pjrt.so")
    )


def kernel(**inputs):
    f = np.float32
    q = np.asarray(inputs["q"], f)
    k = np.asarray(inputs["k"], f)
    v = np.asarray(inputs["v"], f)
    Wq_s = np.asarray(inputs["Wq_s"], f)
    Wk_s = np.asarray(inputs["Wk_s"], f)
    Wv_s = np.asarray(inputs["Wv_s"], f)
    Wq_c = np.asarray(inputs["Wq_c"], f)
    Wk_c = np.asarray(inputs["Wk_c"], f)
    Wv_c = np.asarray(inputs["Wv_c"], f)
    W1 = np.asarray(inputs["W1"], f)
    b1 = np.asarray(inputs["b1"], f)
    W2 = np.asarray(inputs["W2"], f)
    b2 = np.asarray(inputs["b2"], f)
    g1 = np.asarray(inputs["g1"], f)
    be1 = np.asarray(inputs["be1"], f)
    g2 = np.asarray(inputs["g2"], f)
    be2 = np.asarray(inputs["be2"], f)
    g3 = np.asarray(inputs["g3"], f)
    be3 = np.asarray(inputs["be3"], f)

    # ---- host folds ----
    m = q.mean(-1, keepdims=True)
    var = ((q - m) ** 2).mean(-1, keepdims=True)
    xn1 = (q - m) / np.sqrt(var + EPS) * g1 + be1          # [B, L, E]
    M_s = np.einsum("hed,hfd->hef", Wq_s, Wk_s) * SQ
    M_c = np.einsum("hed,hfd->hef", g2[:, None] * Wq_c, Wk_c) * SQ
    beta = np.einsum(
        "hf,blf->bhl", np.einsum("e,hed,hfd->hf", be2, Wq_c, Wk_c) * SQ, k
    ).astype(f)                                             # [B, H, L]
    c1 = be3 @ W1 + b1

    ms8 = _q8(M_s, 8192.0)
    mc8 = _q8(M_c, 8192.0)
    wvs8 = _q8(Wv_s, 1024.0)
    wvc8 = _q8(Wv_c, 1024.0)
    w1u = np.ascontiguousarray((g3[:, None] * W1).astype(BF16))
    w2u = np.ascontiguousarray(W2.astype(BF16))
    b2rep = np.broadcast_to(b2[None, :], (P, E)).astype(f).copy()
    c1s = np.zeros((P, HC), f)
    for c in range(HC):
        c1s[:, c] = c1[c * P:(c + 1) * P]
    ident = np.eye(P, dtype=BF16)
    # tri[key_i, row_j] = 1 where key <= row within the diagonal block
    tri = np.triu(np.ones((P, P), np.float32))

    in_maps = []
    for core in range(8):
        b, rsid = core // 2, core % 2
        S = ROWSETS[rsid]
        rows = np.concatenate([np.arange(s * P, (s + 1) * P) for s in S])
        xn1t = _q8(xn1[b].T, 16.0)
        betac = np.zeros((P, H * KT), f)
        for h in range(H):
            for kt in range(KT):
                betac[:, h * KT + kt] = beta[b, h, kt * P:(kt + 1) * P] + LN8
        # boundary masks: for kt, the block i = kt//2 is full/diag/zero
        mk = np.zeros((P, KT, P), f)
        for kt in range(KT):
            i = kt // 2
            if kt < S[i]:
                mk[:, kt, :] = 1.0
            elif kt == S[i]:
                mk[:, kt, :] = tri
            # else zero
        in_maps.append(
            dict(
                xn1t_full=xn1t,
                xn1t_own=np.ascontiguousarray(xn1t[:, rows]),
                kT=_q8(k[b].T, 16.0),
                vT=_q8(v[b].T, 16.0),
                qres=np.ascontiguousarray(q[b][rows]),
                m_s=ms8, m_c=mc8, wv_s=wvs8, wv_c=wvc8,
                w1u=w1u, w2u=w2u,
                beta=betac, c1s=c1s, b2rep=b2rep,
                identity=ident, masks=mk.astype(FP8),
            )
        )

    if "nc" not in _CACHE:
        _CACHE["nc"] = _build()
    nc = _CACHE["nc"]

    kwargs = {}
    if TRACE:
        _ensure_ntff_hook()
        import os as _os
        _os.environ["BASS_PERFETTO_PROFILE_ALL_CORES"] = "1"
        import tempfile
        kwargs = dict(trace=True, tmpdir=tempfile.mkdtemp())

    res = run_bass_kernel_spmd(nc, in_maps, core_ids=list(range(8)), **kwargs)
    _CACHE["last_res"] = res

    out = np.empty((B, L, E), f)
    for core in range(8):
        b, rsid = core // 2, core % 2
        r = res.results[core]["out"]
        for i, s in enumerate(ROWSETS[rsid]):
            out[b, s * P:(s + 1) * P] = r[i * P:(i + 1) * P]
    return out


# revision 3
# speedup vs baseline: 1.0032x; 1.0032x over previous
"""Trainium2 Bass kernel v4: decoder layer, ROW-parallel SPMD over 8 cores.

Core c = (batch b = c//2, row-set c%2). Row-sets are causal-balanced
interleaved 128-row slices: set0 = {0,3,4,7,8,11,12,15}, set1 = the rest.
Each core computes ALL 8 heads (averaged) for its own 1024 rows -> no
collectives. Host precomputes LN1 and folds Wq@Wk^T per head (E==D) so K
projections vanish; big matmuls run fp8 e4m3 with DoubleRow (contract 256).

One SPMD program for all cores: causal loop bounds are unified by padding
slice i to 2*(i+1) key tiles; the (full / diagonal / invalid) structure of
the boundary tile kt is applied as a data-driven mask (ones / tri / zero)
uploaded per core, one mask multiply per key tile.

Scales (powers of 2, folded into activation scale/bias):
  xn1/k/v/z fp8 = 16*x ; M~ = 8192*M ; q_t = 1024*(z@M) (evac 1/128)
  scores psum = 16384*s ; exp bias += ln(8) -> p8 = 8*exp(s) fp8
  Wv~ = 1024*Wv ; v_t = 16*V (evac 1/1024) ; rowsum ones = 128
  po/pr = sum(p V)/(8*rowsum) = the per-head mean term, exactly.
  W1u = 1024*(g3 W1) ; h1 = 16*relu(.) (evac 1/1024, bias 16*c1)
  W2u = 1024*W2 ; h2 = psum/16384.
"""
import sys
import types

sys.path.insert(0, "/opt/trn_rl_repo")

import numpy as np
import ml_dtypes

import concourse.bacc as bacc
import concourse.tile as tile
from concourse import mybir
from concourse.bass_utils import run_bass_kernel_spmd

FP8 = ml_dtypes.float8_e4m3
BF16 = ml_dtypes.bfloat16
B, L, E, H, D, FW = 4, 2048, 512, 8, 512, 4
HID = FW * E
P = 128
EC = E // P          # 4
KT = L // P          # 16
HC = HID // P        # 16
OWN = 8              # own row slices per core
OWNL = OWN * P       # 1024
SQ = float(D) ** -0.5
EPS = 1e-5
LN8 = float(np.log(8.0))
ROWSETS = [[0, 3, 4, 7, 8, 11, 12, 15], [1, 2, 5, 6, 9, 10, 13, 14]]

TRACE = False
_CACHE = {}
DR = mybir.MatmulPerfMode.DoubleRow


def _build():
    f32, bf16, fp8 = mybir.dt.float32, mybir.dt.bfloat16, mybir.dt.float8e4
    nc = bacc.Bacc(None, target_bir_lowering=False, debug=False)

    xf_in = nc.dram_tensor("xn1t_full", [E, L], fp8, kind="ExternalInput")
    xo_in = nc.dram_tensor("xn1t_own", [E, OWNL], fp8, kind="ExternalInput")
    kt_in = nc.dram_tensor("kT", [E, L], fp8, kind="ExternalInput")
    vt_in = nc.dram_tensor("vT", [E, L], fp8, kind="ExternalInput")
    qr_in = nc.dram_tensor("qres", [OWNL, E], f32, kind="ExternalInput")
    ms_in = nc.dram_tensor("m_s", [H, E, E], fp8, kind="ExternalInput")
    mc_in = nc.dram_tensor("m_c", [H, E, E], fp8, kind="ExternalInput")
    wvs_in = nc.dram_tensor("wv_s", [H, E, D], fp8, kind="ExternalInput")
    wvc_in = nc.dram_tensor("wv_c", [H, E, D], fp8, kind="ExternalInput")
    w1_in = nc.dram_tensor("w1u", [E, HID], bf16, kind="ExternalInput")
    w2_in = nc.dram_tensor("w2u", [HID, E], bf16, kind="ExternalInput")
    beta_in = nc.dram_tensor("beta", [P, H * KT], f32, kind="ExternalInput")
    c1_in = nc.dram_tensor("c1s", [P, HC], f32, kind="ExternalInput")
    b2_in = nc.dram_tensor("b2rep", [P, E], f32, kind="ExternalInput")
    id_in = nc.dram_tensor("identity", [P, P], bf16, kind="ExternalInput")
    mk_in = nc.dram_tensor("masks", [P, KT, P], fp8, kind="ExternalInput")
    out_d = nc.dram_tensor("out", [OWNL, E], f32, kind="ExternalOutput")

    with tile.TileContext(nc) as tc:
        with (
            tc.tile_pool(name="consts", bufs=1) as consts,
            tc.tile_pool(name="xin", bufs=1) as xin,
            tc.tile_pool(name="accp", bufs=1) as accp,
            tc.tile_pool(name="wpool", bufs=2) as wpool,
            tc.tile_pool(name="ffw", bufs=1) as ffw,
            tc.tile_pool(name="qtp", bufs=2) as qtp,
            tc.tile_pool(name="vtp", bufs=3) as vtp,
            tc.tile_pool(name="ptp", bufs=2) as ptp,
            tc.tile_pool(name="ztp", bufs=1) as ztp,
            tc.tile_pool(name="h1p", bufs=1) as h1p,
            tc.tile_pool(name="stats", bufs=6) as statp,
            tc.tile_pool(name="tmps", bufs=3) as tmpp,
            tc.tile_pool(name="ps_big", bufs=2, space="PSUM") as psb,
            tc.tile_pool(name="ps_sc", bufs=2, space="PSUM") as psc,
            tc.tile_pool(name="ps_sum", bufs=1, space="PSUM") as pss,
            tc.tile_pool(name="ps_tr", bufs=1, space="PSUM") as pstr,
        ):
            # inputs needed first (order sets DMA priority)
            xft = xin.tile([P, EC, L], fp8)
            nc.sync.dma_start(
                xft[:, :, 0:256],
                xf_in.rearrange("(c p) t -> p c t", p=P)[:, :, 0:256])
            nc.sync.dma_start(
                xft[:, :, 256:1024],
                xf_in.rearrange("(c p) t -> p c t", p=P)[:, :, 256:1024])
            xot = xin.tile([P, EC, OWNL], fp8)
            nc.sync.dma_start(xot, xo_in.rearrange("(c p) t -> p c t", p=P))
            ones2 = consts.tile([P, 2, 1], fp8)
            nc.vector.memset(ones2, 128.0)
            ln8t = consts.tile([P, 1], f32)
            nc.vector.memset(ln8t, LN8)
            epsA = consts.tile([P, 1], f32)
            nc.vector.memset(epsA, EPS)
            epsB = consts.tile([P, 1], f32)
            nc.vector.memset(epsB, EPS / 256.0)
            nc.sync.dma_start(
                xft[:, :, 1024:2048],
                xf_in.rearrange("(c p) t -> p c t", p=P)[:, :, 1024:2048])

            acc = accp.tile([P, OWN, E], f32)
            z2T = ztp.tile([P, EC, OWNL], fp8, tag="z2")
            z3T = ztp.tile([P, EC, OWNL], bf16, tag="z3")

            def ln_to_zT(src_ap, zT, col, s16=True):
                st = statp.tile([P, 6], f32, tag="bnst")
                nc.vector.bn_stats(st, src_ap)
                mv = statp.tile([P, 2], f32, tag="bnmv")
                nc.vector.bn_aggr(mv, st)
                sd = statp.tile([P, 1], f32, tag="bnsd")
                nc.scalar.activation(
                    sd, mv[:, 1:2], mybir.ActivationFunctionType.Sqrt,
                    bias=epsB if s16 else epsA,
                    scale=1.0 / 256.0 if s16 else 1.0,
                )
                r16 = statp.tile([P, 1], f32, tag="bnrs")
                nc.vector.reciprocal(r16, sd)
                xn = tmpp.tile([P, E], bf16, tag="bfs")
                nc.vector.tensor_scalar(
                    out=xn, in0=src_ap, scalar1=mv[:, 0:1], scalar2=r16,
                    op0=mybir.AluOpType.subtract, op1=mybir.AluOpType.mult,
                )
                for dt in range(EC):
                    pt = pstr.tile([P, P], bf16, tag="ptr")
                    nc.tensor.transpose(pt, xn[:, dt * P:(dt + 1) * P], idt)
                    nc.scalar.activation(
                        zT[:, dt, col:col + P], pt,
                        mybir.ActivationFunctionType.Copy,
                    )

            def vproj_kt(kt, vkeyT, wv, v_t):
                ps = psb.tile([P, 512], f32, tag="psb")
                for c2 in range(EC // 2):
                    nc.tensor.matmul(
                        ps,
                        vkeyT[:, 2 * c2:2 * c2 + 2, kt * P:(kt + 1) * P],
                        wv[:, 2 * c2:2 * c2 + 2, :],
                        start=(c2 == 0), stop=(c2 == EC // 2 - 1),
                        perf_mode=DR,
                    )
                nc.vector.tensor_scalar_mul(v_t[:, kt, :], ps, 1.0 / 1024.0)

            def vproj_weights(h, wv_d):
                wv = wpool.tile([P, EC, D], fp8, tag="wv")
                nc.sync.dma_start(wv, wv_d[h].rearrange("(c p) d -> p c d", p=P))
                return wv

            def projq_units(h, xqT, m_d):
                """Thunk list: 8 psum groups of the Q~ projection + DVE evacs."""
                wm = wpool.tile([P, EC, E], fp8, tag="wm")
                nc.sync.dma_start(wm, m_d[h].rearrange("(c p) e -> p c e", p=P))
                q_t = qtp.tile([P, EC, OWNL], fp8, tag="qt")

                def unit(rb, e2):
                    def f():
                        ps = psb.tile([P, 512], f32, tag="psb")
                        for c2 in range(EC // 2):
                            nc.tensor.matmul(
                                ps,
                                wm[:, 2 * c2:2 * c2 + 2, e2 * P:(e2 + 1) * P],
                                xqT[:, 2 * c2:2 * c2 + 2,
                                    rb * 512:(rb + 1) * 512],
                                start=(c2 == 0), stop=(c2 == EC // 2 - 1),
                                perf_mode=DR,
                            )
                        nc.vector.tensor_scalar_mul(
                            q_t[:, e2, rb * 512:(rb + 1) * 512], ps, 1.0 / 128.0)
                    return f
                return q_t, [unit(rb, e2) for rb in range(2) for e2 in range(EC)]

            def vproj_units(h, vkeyT, wv_d):
                wv = vproj_weights(h, wv_d)
                v_t = vtp.tile([P, KT, D], fp8, tag="vt")
                return v_t, [
                    (lambda kt: lambda: vproj_kt(kt, vkeyT, wv, v_t))(kt)
                    for kt in range(KT)
                ]

            def sc_units(h, q_t, keyT, causal, bcol):
                p_t = ptp.tile([P, KT, OWNL], fp8, tag="pt")

                def unit(kt):
                    def f():
                        c0 = (kt // 2) * P if causal else 0
                        n = OWNL - c0
                        ps = psc.tile([P, 1024], f32, tag="psc")
                        for cs in range(c0, OWNL, 512):
                            ce = min(cs + 512, OWNL)
                            for c2 in range(EC // 2):
                                nc.tensor.matmul(
                                    ps[:, cs - c0:ce - c0],
                                    keyT[:, 2 * c2:2 * c2 + 2,
                                         kt * P:(kt + 1) * P],
                                    q_t[:, 2 * c2:2 * c2 + 2, cs:ce],
                                    start=(c2 == 0), stop=(c2 == EC // 2 - 1),
                                    perf_mode=DR,
                                )
                        bias = (
                            betat[:, bcol + kt:bcol + kt + 1]
                            if bcol is not None else ln8t
                        )
                        nc.scalar.activation(
                            p_t[:, kt, c0:OWNL], ps[:, 0:n],
                            mybir.ActivationFunctionType.Exp,
                            bias=bias, scale=1.0 / 16384.0,
                        )
                        if causal:
                            i = kt // 2
                            nc.gpsimd.tensor_mul(
                                p_t[:, kt, i * P:(i + 1) * P],
                                p_t[:, kt, i * P:(i + 1) * P],
                                masks[:, kt, :],
                            )
                    return f

                return p_t, [unit(kt) for kt in range(KT)]

            def emit_interleaved(a, b):
                """Emit thunks from a and b, spreading b evenly among a."""
                if not b:
                    for f in a:
                        f()
                    return
                ratio = max(1, len(a) // len(b))
                bi = 0
                for n, f in enumerate(a):
                    f()
                    if n % ratio == ratio - 1 and bi < len(b):
                        b[bi]()
                        bi += 1
                while bi < len(b):
                    b[bi]()
                    bi += 1

            def av(p_t, v_t, causal, post_slice=None):
                for i in range(OWN):
                    pairs = (i + 1) if causal else KT // 2
                    cs = slice(i * P, (i + 1) * P)
                    pr = pss.tile([P, 1], f32, tag="pssum")
                    for j in range(pairs):
                        nc.tensor.matmul(
                            pr, p_t[:, 2 * j:2 * j + 2, cs], ones2[:, :, :],
                            start=(j == 0), stop=(j == pairs - 1),
                            perf_mode=DR,
                        )
                    r8 = statp.tile([P, 1], f32, tag="r8")
                    nc.vector.reciprocal(r8, pr)
                    po = psb.tile([P, 512], f32, tag="psb")
                    for j in range(pairs):
                        nc.tensor.matmul(
                            po, p_t[:, 2 * j:2 * j + 2, cs],
                            v_t[:, 2 * j:2 * j + 2, :],
                            start=(j == 0), stop=(j == pairs - 1),
                            perf_mode=DR,
                        )
                    ot = tmpp.tile([P, E], f32, tag="f32s")
                    nc.vector.tensor_scalar_mul(ot, po, r8)
                    nc.gpsimd.tensor_add(acc[:, i, :], acc[:, i, :], ot)
                    if post_slice is not None:
                        post_slice(i)

            # ================= self-attention, software-pipelined =================
            v0, vu0 = vproj_units(0, xft, wvs_in)
            q0, qu0 = projq_units(0, xot, ms_in)
            emit_interleaved(vu0, [])
            emit_interleaved(qu0, [])
            # constants not needed in the first ~10us
            idt = consts.tile([P, P], bf16)
            nc.sync.dma_start(idt, id_in[:, :])
            masks = consts.tile([P, KT, P], fp8)
            nc.sync.dma_start(masks, mk_in[:, :, :])
            betat = consts.tile([P, H * KT], f32)
            nc.sync.dma_start(betat, beta_in[:, :])
            c1t = consts.tile([P, HC], f32)
            nc.sync.dma_start(c1t, c1_in[:, :])
            b2t = consts.tile([P, E], f32)
            nc.sync.dma_start(b2t, b2_in[:, :])
            nc.sync.dma_start(acc, qr_in.rearrange("(s p) e -> p s e", p=P))

            v_prev, q_prev = v0, q0
            p_prev = None
            for h in range(H):
                # interleave scores(h) with next head's projections
                pu_next = []
                if h + 1 < H:
                    v_next, vun = vproj_units(h + 1, xft, wvs_in)
                    q_next, qun = projq_units(h + 1, xot, ms_in)
                    pu_next = vun + qun
                else:
                    # last self head: overlap with cross head0 V-projection
                    v_next, vun = vproj_units(0, vtt, wvc_in)
                    q_next = None
                    pu_next = vun
                p_h, scu = sc_units(h, q_prev, xft, True, None)
                emit_interleaved(scu, pu_next)
                if p_prev is not None:
                    av(p_prev, vv_prev, True)
                p_prev, vv_prev = p_h, v_prev
                v_prev, q_prev = v_next, q_next
                if h == 1:
                    ktt = xin.tile([P, EC, L], fp8)
                    nc.sync.dma_start(
                        ktt, kt_in.rearrange("(c p) t -> p c t", p=P))
                    vtt = xin.tile([P, EC, L], fp8)
                    nc.sync.dma_start(
                        vtt, vt_in.rearrange("(c p) t -> p c t", p=P))
                elif h == 2:
                    w1t = ffw.tile([P, EC, HID], bf16)
                    nc.sync.dma_start(
                        w1t, w1_in.rearrange("(c p) d -> p c d", p=P))
                    w2t = ffw.tile([P, HC, E], bf16)
                    nc.sync.dma_start(
                        w2t, w2_in.rearrange("(c p) d -> p c d", p=P))
            av(p_prev, vv_prev, True)   # self head 7

            # ====== LN2 -> z2T, interleaved with cross head1 V-proj ======
            vc1, vuc1 = vproj_units(1, vtt, wvc_in)
            for i in range(OWN):
                ln_to_zT(acc[:, i, :], z2T, i * P)
                for j in range(2 * i, 2 * i + 2):
                    vuc1[j]()

            # ================= cross-attention, software-pipelined =================
            vs = {0: v_prev, 1: vc1}          # v_prev = cross head0 v_t
            q_prev, _qu = projq_units(0, z2T, mc_in)
            emit_interleaved(_qu, [])
            p_prev = None
            vv_prev = None
            for h in range(H):
                pu_next = []
                if h + 1 < H:
                    if h >= 1 and h + 1 not in vs:
                        vs[h + 1], vun = vproj_units(h + 1, vtt, wvc_in)
                        pu_next += vun
                    q_next, qun = projq_units(h + 1, z2T, mc_in)
                    pu_next += qun
                else:
                    q_next = None
                p_h, scu = sc_units(h, q_prev, ktt, False, h * KT)
                emit_interleaved(scu, pu_next)
                if p_prev is not None:
                    av(p_prev, vv_prev, False)
                p_prev, vv_prev = p_h, vs[h]
                q_prev = q_next

            # last cross AV with LN3 + b2 interleaved per slice
            def post_ln3(i):
                ln_to_zT(acc[:, i, :], z3T, i * P, s16=False)
                nc.vector.tensor_add(acc[:, i, :], acc[:, i, :], b2t)
            av(p_prev, vv_prev, False, post_slice=post_ln3)

            # ================= FFN per row-block =================
            for rb in range(2):
                h1t = h1p.tile([P, HC, 512], bf16, tag="h1")
                for hc in range(HC):
                    ps = psb.tile([P, 512], f32, tag="psb")
                    for c2 in range(EC):
                        nc.tensor.matmul(
                            ps,
                            w1t[:, c2, hc * P:(hc + 1) * P],
                            z3T[:, c2, rb * 512:(rb + 1) * 512],
                            start=(c2 == 0), stop=(c2 == EC - 1),
                        )
                    nc.scalar.activation(
                        h1t[:, hc, :], ps,
                        mybir.ActivationFunctionType.Relu,
                        bias=c1t[:, hc:hc + 1],
                    )
                for i in range(rb * 4, rb * 4 + 4):
                    io = i - rb * 4
                    ps = psb.tile([P, 512], f32, tag="psb")
                    for j in range(HC):
                        nc.tensor.matmul(
                            ps,
                            h1t[:, j, io * P:(io + 1) * P],
                            w2t[:, j, :],
                            start=(j == 0), stop=(j == HC - 1),
                        )
                    t1 = tmpp.tile([P, E], f32, tag="f32s")
                    nc.scalar.activation(
                        t1, ps, mybir.ActivationFunctionType.Copy,
                    )
                    t2 = tmpp.tile([P, E], f32, tag="f32o")
                    nc.vector.tensor_add(t2, t1, acc[:, i, :])
                    nc.sync.dma_start(out_d[i * P:(i + 1) * P, :], t2)

    nc.compile()
    return nc


def _q8(x, scale=1.0):
    return np.ascontiguousarray(
        np.clip(np.asarray(x, np.float32) * scale, -240, 240).astype(FP8)
    )


def _ensure_ntff_hook():
    try:
        from antenv.axon_hooks import get_axon_ntff_profile_hook  # noqa: F401
        return
    except ImportError:
        pass
    import antenv

    mod = types.ModuleType("antenv.axon_hooks")
    _hook = [None]
    mod.set_axon_ntff_profile_hook = lambda h: _hook.__setitem__(0, h)
    mod.get_axon_ntff_profile_hook = lambda: _hook[0]
    sys.modules["antenv.axon_hooks"] = mod
    antenv.axon_hooks = mod
    from trn_agent_boot.trn_boot import _ntff_profile_via_ctypes

    mod.set_axon_ntff_profile_hook(
        _ntff_profile_via_ctypes("/opt/axon/libaxon_pjrt.so")
    )


def kernel(**inputs):
    f = np.float32
    q = np.asarray(inputs["q"], f)
    k = np.asarray(inputs["k"], f)
    v = np.asarray(inputs["v"], f)
    Wq_s = np.asarray(inputs["Wq_s"], f)
    Wk_s = np.asarray(inputs["Wk_s"], f)
    Wv_s = np.asarray(inputs["Wv_s"], f)
    Wq_c = np.asarray(inputs["Wq_c"], f)
    Wk_c = np.asarray(inputs["Wk_c"], f)
    Wv_c = np.asarray(inputs["Wv_c"], f)
    W1 = np.asarray(inputs["W1"], f)
    b1 = np.asarray(inputs["b1"], f)
    W2 = np.asarray(inputs["W2"], f)
    b2 = np.asarray(inputs["b2"], f)
    g1 = np.asarray(inputs["g1"], f)
    be1 = np.asarray(inputs["be1"], f)
    g2 = np.asarray(inputs["g2"], f)
    be2 = np.asarray(inputs["be2"], f)
    g3 = np.asarray(inputs["g3"], f)
    be3 = np.asarray(inputs["be3"], f)

    # ---- host folds ----
    m = q.mean(-1, keepdims=True)
    var = ((q - m) ** 2).mean(-1, keepdims=True)
    xn1 = (q - m) / np.sqrt(var + EPS) * g1 + be1          # [B, L, E]
    M_s = np.einsum("hed,hfd->hef", Wq_s, Wk_s) * SQ
    M_c = np.einsum("hed,hfd->hef", g2[:, None] * Wq_c, Wk_c) * SQ
    beta = np.einsum(
        "hf,blf->bhl", np.einsum("e,hed,hfd->hf", be2, Wq_c, Wk_c) * SQ, k
    ).astype(f)                                             # [B, H, L]
    c1 = be3 @ W1 + b1

    ms8 = _q8(M_s, 8192.0)
    mc8 = _q8(M_c, 8192.0)
    wvs8 = _q8(Wv_s, 1024.0)
    wvc8 = _q8(Wv_c, 1024.0)
    w1u = np.ascontiguousarray((g3[:, None] * W1).astype(BF16))
    w2u = np.ascontiguousarray(W2.astype(BF16))
    b2rep = np.broadcast_to(b2[None, :], (P, E)).astype(f).copy()
    c1s = np.zeros((P, HC), f)
    for c in range(HC):
        c1s[:, c] = c1[c * P:(c + 1) * P]
    ident = np.eye(P, dtype=BF16)
    # tri[key_i, row_j] = 1 where key <= row within the diagonal block
    tri = np.triu(np.ones((P, P), np.float32))

    in_maps = []
    for core in range(8):
        b, rsid = core // 2, core % 2
        S = ROWSETS[rsid]
        rows = np.concatenate([np.arange(s * P, (s + 1) * P) for s in S])
        xn1t = _q8(xn1[b].T, 16.0)
        betac = np.zeros((P, H * KT), f)
        for h in range(H):
            for kt in range(KT):
                betac[:, h * KT + kt] = beta[b, h, kt * P:(kt + 1) * P] + LN8
        # boundary masks: for kt, the block i = kt//2 is full/diag/zero
        mk = np.zeros((P, KT, P), f)
        for kt in range(KT):
            i = kt // 2
            if kt < S[i]:
                mk[:, kt, :] = 1.0
            elif kt == S[i]:
                mk[:, kt, :] = tri
            # else zero
        in_maps.append(
            dict(
                xn1t_full=xn1t,
                xn1t_own=np.ascontiguousarray(xn1t[:, rows]),
                kT=_q8(k[b].T, 16.0),
                vT=_q8(v[b].T, 16.0),
                qres=np.ascontiguousarray(q[b][rows]),
                m_s=ms8, m_c=mc8, wv_s=wvs8, wv_c=wvc8,
                w1u=w1u, w2u=w2u,
                beta=betac, c1s=c1s, b2rep=b2rep,
                identity=ident, masks=mk.astype(FP8),
            )
        )

    if "nc" not in _CACHE:
        _CACHE["nc"] = _build()
    nc = _CACHE["nc"]

    kwargs = {}
    if TRACE:
        _ensure_ntff_hook()
        import os as _os
        _os.environ["BASS_PERFETTO_PROFILE_ALL_CORES"] = "1"
        import tempfile
        kwargs = dict(trace=True, tmpdir=tempfile.mkdtemp())

    res = run_bass_kernel_spmd(nc, in_maps, core_ids=list(range(8)), **kwargs)
    _CACHE["last_res"] = res

    out = np.empty((B, L, E), f)
    for core in range(8):
        b, rsid = core // 2, core % 2
        r = res.results[core]["out"]
        for i, s in enumerate(ROWSETS[rsid]):
            out[b, s * P:(s + 1) * P] = r[i * P:(i + 1) * P]
    return out
